# revision 31
# baseline (speedup 1.0000x reference)
"""GroupedQueryAttention TRN2 kernel (bf16).

Sharding: 4-way tensor-parallel over heads x 2-way data-parallel over batch.
Core c handles batch b=c//4 and head-group gc=c%4 (kv heads {2gc, 2gc+1},
q heads (hkv, g) for g in 0..3 -> 8 q heads per core).

Per-core device program (T=2048, C=2048, D=64), all matmul operands bf16
(PSUM accumulation fp32):
  phase A (PE-bound ~82us): Q^T/K^T/V^T projections per 512-col block,
    V PE-transposed into V' [T,130] with ones cols (rowsum trick).
  phase B (Scalar exp-bound): S^T tiles = K_tile Q^T (causal widths),
    exp on ScalarE -> bf16, diag mask on GpSimd, O'^T accumulated on PE
    (row 64 = rowsum), normalize via ones2-matmul broadcast of rowsums +
    DVE reciprocal_approx_fast + DVE multiply.
  phase C (PE-bound): y_tile = O^T.T @ Wo_g, interleaved INTO phase B at
    (j,g) boundaries so PE fills exp-wait stalls with output-proj matmuls.
  Host sums the 4 TP partials per batch in fp32.

PSUM budget in B||C: ps 2x2 banks + po 2x1 + py/pru 2x1 = 8 banks.
"""

import numpy as np
import ml_dtypes

import concourse.bass as bass
import concourse.mybir as mybir
import concourse.tile as tile
from concourse import bacc
from concourse.bass_utils import run_bass_kernel_spmd

H, HKV, D, G = 32, 8, 64, 4
B, T, C = 2, 2048, 2048
P = 128
NCORES = 8
F32 = mybir.dt.float32
BF = mybir.dt.bfloat16
BFNP = ml_dtypes.bfloat16

NT = T // 512   # 4 q blocks of 512
NK = C // P     # 16 contraction tiles
NTT = T // P    # 16 key/row tiles

_NC_CACHE = None


def build_kernel(nc, tc, ins, outs):
    xTr = ins["xT"].rearrange("(ko p) t -> p ko t", p=P)
    y = outs["y"]
    EXP = mybir.ActivationFunctionType.Exp
    CPY = mybir.ActivationFunctionType.Copy

    # ---- persistent SBUF ----
    persist = tc._persist_pool
    qT_sb = persist.tile([P, G, T], BF, name="qT_sb", tag="qT_sb")
    kT_sb = persist.tile([P, T], BF, name="kT_sb", tag="kT_sb")
    v_sb = persist.tile([P, NTT, 130], BF, name="v_sb", tag="v_sb")
    oT_sb = persist.tile([P, G, T], BF, name="oT_sb", tag="oT_sb")
    wo_sb = persist.tile([P, G, C], BF, name="wo_sb", tag="wo_sb")
    tri_sb = persist.tile([P, P], BF, name="tri_sb", tag="tri_sb")
    iden_sb = persist.tile([P, P], BF, name="iden_sb", tag="iden_sb")
    ones2_sb = persist.tile([2, P], BF, name="ones2_sb", tag="ones2_sb")

    def emit_const_dmas():
        # deferred behind the first weight/x chunks: only needed ~20us in
        nc.sync.dma_start(tri_sb[:], ins["tri"][:])
        nc.sync.dma_start(iden_sb[:], ins["iden"][:])
        nc.sync.dma_start(ones2_sb[:], ins["ones2"][:])
        # ones columns of V' (64, 129 per key tile); data cols overwritten
        nc.sync.dma_start(v_sb[:], ins["vinit"][:])

    # ================= phase A: projections =================
    with (
        tc.tile_pool(name="wproj", bufs=1) as wpool,
        tc.tile_pool(name="xt", bufs=12) as xpool,
        tc.tile_pool(name="vtt", bufs=2) as vttpool,
        tc.tile_pool(name="pp", bufs=6, space="PSUM") as pp,
        tc.tile_pool(name="pvt", bufs=2, space="PSUM") as pvt,
    ):
        # weights and x in k-tile chunks as separate tiles: fine-grained
        # deps let the k=0 matmuls start after ~0.26 MB of DMA, not ~6 MB.
        # chunk c covers k tiles KCH[c]..KCH[c+1]
        KCH = [0, 1, 2, 4, 8, 12, 16]
        NCH = len(KCH) - 1
        kch = lambda k: next(c for c in range(NCH) if KCH[c + 1] > k)
        wq_t = [wpool.tile([P, KCH[c + 1] - KCH[c], 512], BF, name=f"wq_{c}")
                for c in range(NCH)]
        wk_sb = wpool.tile([P, NK, P], BF, name="wk_sb")
        wv_sb = wpool.tile([P, NK, P], BF, name="wv_sb")
        wqr = ins["wqT"].rearrange("(ko p) m -> p ko m", p=P)
        xt0 = [xpool.tile([P, KCH[c + 1] - KCH[c], 512], BF, tag="xt",
                          name=f"xt_0_{c}") for c in range(NCH)]
        for c in range(NCH):
            ks = slice(KCH[c], KCH[c + 1])
            nc.sync.dma_start(wq_t[c][:], wqr[:, ks, :])
            nc.sync.dma_start(xt0[c][:], xTr[:, ks, 0:512])
            if c == 2:
                nc.sync.dma_start(wk_sb[:],
                                  ins["wkT"].rearrange("(ko p) m -> p ko m", p=P))
                nc.sync.dma_start(wv_sb[:],
                                  ins["wvT"].rearrange("(ko p) m -> p ko m", p=P))
            if c == 3:
                emit_const_dmas()

        for tb in range(NT):
            ts = slice(tb * 512, (tb + 1) * 512)
            if tb == 0:
                xts = xt0
            else:
                xts = [xpool.tile([P, KCH[c + 1] - KCH[c], 512], BF, tag="xt",
                                  name=f"xt_{tb}_{c}") for c in range(NCH)]
                for c in range(NCH):
                    nc.sync.dma_start(xts[c][:], xTr[:, KCH[c]:KCH[c + 1], ts])
            psq = [pp.tile([P, 512], F32, tag="pp", name=f"psq_{tb}_{g}")
                   for g in range(G)]
            psk = pp.tile([P, 512], F32, tag="pp", name=f"psk_{tb}")
            psv = pp.tile([P, 512], F32, tag="pp", name=f"psv_{tb}")
            if tb == 0:
                # Q matmuls first: the in-order PE queue would otherwise
                # head-of-line block on psk/psv waiting for the wk/wv DMAs.
                for k in range(NK):
                    c = kch(k)
                    for g in range(G):
                        nc.tensor.matmul(psq[g][:],
                                         wq_t[c][:, k - KCH[c], g * P:(g + 1) * P],
                                         xts[c][:, k - KCH[c], :],
                                         start=(k == 0), stop=(k == NK - 1))
                for k in range(NK):
                    st, sp = (k == 0), (k == NK - 1)
                    c = kch(k)
                    xk = xts[c][:, k - KCH[c], :]
                    nc.tensor.matmul(psk[:], wk_sb[:, k, :], xk,
                                     start=st, stop=sp)
                    nc.tensor.matmul(psv[:], wv_sb[:, k, :], xk,
                                     start=st, stop=sp)
            else:
                for k in range(NK):
                    st, sp = (k == 0), (k == NK - 1)
                    c = kch(k)
                    xk = xts[c][:, k - KCH[c], :]
                    for g in range(G):
                        nc.tensor.matmul(psq[g][:],
                                         wq_t[c][:, k - KCH[c], g * P:(g + 1) * P],
                                         xk, start=st, stop=sp)
                    nc.tensor.matmul(psk[:], wk_sb[:, k, :], xk,
                                     start=st, stop=sp)
                    nc.tensor.matmul(psv[:], wv_sb[:, k, :], xk,
                                     start=st, stop=sp)
            vtt = vttpool.tile([P, 512], BF, tag="vtt", name=f"vtt_{tb}")
            # copies split Scalar/DVE to shorten the per-tb tail
            nc.scalar.activation(qT_sb[:, 0, ts], psq[0][:], CPY)
            nc.scalar.activation(qT_sb[:, 1, ts], psq[1][:], CPY)
            nc.scalar.activation(kT_sb[:, ts], psk[:], CPY)
            nc.vector.tensor_copy(qT_sb[:, 2, ts], psq[2][:])
            nc.vector.tensor_copy(qT_sb[:, 3, ts], psq[3][:])
            nc.vector.tensor_copy(vtt[:], psv[:])
            # V^T -> V (PE transpose) into V' layout with ones cols
            for dd in range(4):
                tt = 4 * tb + dd
                pt_ = pvt.tile([P, P], BF, tag="pvt", name=f"pvt_{tt}")
                nc.tensor.transpose(pt_[:], vtt[:, dd * P:(dd + 1) * P],
                                    iden_sb[:])
                nc.vector.tensor_copy(v_sb[:, tt, 0:64], pt_[:, 0:64])
                nc.vector.tensor_copy(v_sb[:, tt, 65:129], pt_[:, 64:128])

    # ================= phase B || C =================
    for c4 in range(4):
        nc.sync.dma_start(
            wo_sb[:, c4, :],
            ins["woR"][128 * c4:128 * (c4 + 1), :].rearrange(
                "(m p) c -> p m c", p=P))

    with (
        tc.tile_pool(name="ps", bufs=2, space="PSUM") as pspool,
        tc.tile_pool(name="po", bufs=2, space="PSUM") as popool,
        tc.tile_pool(name="py", bufs=2, space="PSUM") as pypool,
        tc.tile_pool(name="ptp", bufs=6) as ptpool,
        tc.tile_pool(name="rs", bufs=8) as rspool,
        tc.tile_pool(name="rb", bufs=2) as rbpool,
        tc.tile_pool(name="ysb", bufs=2) as ypool,
    ):
        def emit_B_tiles(j, g, pool=None):
            """Scores + exp + mask + PV accumulation; returns po + rowsum
            copies (issued immediately so DVE overlaps the following C
            chunk)."""
            qs0 = j * 512
            pool, ptag = pool or (popool, "po")
            po = [pool.tile([P, 512], F32, tag=ptag, name=f"po_{j}_{g}_{h}")
                  for h in range(2)]
            ilast = 4 * j + 3
            for i in range(4 * j + 4):
                loc = max(0, P * i - qs0)
                ps = pspool.tile([P, 2, 512], F32, tag="ps",
                                 name=f"ps_{j}_{g}_{i}")
                pt_ = ptpool.tile([P, 2, 512], BF, tag="pt",
                                  name=f"pt_{j}_{g}_{i}")
                for h in range(2):
                    nc.tensor.matmul(
                        ps[:, h, loc:512],
                        kT_sb[h * 64:(h + 1) * 64, i * P:(i + 1) * P],
                        qT_sb[h * 64:(h + 1) * 64, g, qs0 + loc:qs0 + 512],
                        start=True, stop=True,
                    )
                nc.scalar.activation(pt_[:, :, loc:512], ps[:, :, loc:512],
                                     EXP, scale=0.125)
                if i >= 4 * j:  # diagonal tile: mask strict-lower triangle
                    nc.vector.tensor_mul(
                        pt_[:, :, loc:loc + P], pt_[:, :, loc:loc + P],
                        tri_sb[:, None, :].to_broadcast([P, 2, P]),
                    )
                for h in range(2):
                    nc.tensor.matmul(
                        po[h][0:65, loc:512],
                        v_sb[:, i, h * 65:h * 65 + 65],
                        pt_[:, h, loc:512],
                        start=(i == 0), stop=(i == ilast),
                    )
            rs = [rspool.tile([1, 512], BF, tag="rs", name=f"rs_{j}_{g}_{h}")
                  for h in range(2)]
            for h in range(2):
                nc.vector.tensor_copy(rs[h][:], po[h][64:65, :])
            return po, rs

        def emit_B_norm(j, g, po, rs, pool=None):
            """Broadcast rowsums (PE), reciprocal + normalize (DVE).
            Emitted after the interleaved C chunk so PE never waits on the
            DVE rowsum copies."""
            qs0 = j * 512
            pool, ptag = pool or (pypool, "py")
            pru = pool.tile([P, 512], F32, tag=ptag, name=f"pru_{j}_{g}")
            for h in range(2):
                nc.tensor.matmul(pru[h * 64:(h + 1) * 64, :], ones2_sb[0:1, 0:64],
                                 rs[h][:], start=True, stop=True)
            rb = rbpool.tile([P, 512], F32, tag="rb", name=f"rb_{j}_{g}")
            nc.vector.reciprocal_approx_fast(rb[:], pru[:])
            for h in range(2):
                nc.vector.tensor_mul(
                    oT_sb[h * 64:(h + 1) * 64, g, qs0:qs0 + 512],
                    po[h][0:64, :], rb[h * 64:(h + 1) * 64, :],
                )

        def emit_C_tt(tt, final=False):
            ysb = ypool.tile([P, T], BF, tag="ysb", name=f"y_{tt}")
            for cb in range(NT):
                py = pypool.tile([P, 512], F32, tag="py", name=f"py_{tt}_{cb}")
                for m in range(G):
                    nc.tensor.matmul(
                        py[:], oT_sb[:, m, tt * P:(tt + 1) * P],
                        wo_sb[:, m, cb * 512:(cb + 1) * 512],
                        start=(m == 0), stop=(m == G - 1),
                    )
                dst = ysb[:, cb * 512:(cb + 1) * 512]
                if final and cb % 2 == 0:
                    # drain phase: Scalar is idle, split copies + chunk DMA
                    nc.scalar.activation(dst, py[:], CPY)
                else:
                    nc.vector.tensor_copy(dst, py[:])
                if final and cb % 2 == 1:
                    nc.sync.dma_start(
                        y[tt * P:(tt + 1) * P, (cb - 1) * 512:(cb + 1) * 512],
                        ysb[:, (cb - 1) * 512:(cb + 1) * 512])
            if not final:
                nc.sync.dma_start(y[tt * P:(tt + 1) * P, :], ysb[:])

        # Normalize is delayed by one group: norm(prev) is emitted after
        # tiles(cur), so its rowsum copies (DVE) always finish while PE
        # runs the current group's score matmuls -- the broadcast matmul
        # never waits. C(j-1) chunks fill each (j,g) boundary.
        prev = None
        for j in range(NT):
            for g in range(G):
                cur = (j, g) + emit_B_tiles(j, g)
                if prev is not None:
                    emit_B_norm(*prev)
                if j >= 1:
                    emit_C_tt(4 * (j - 1) + g)
                prev = cur
        emit_B_norm(*prev)
        for dd in range(4):
            emit_C_tt(12 + dd, final=True)


def build_nc():
    global _NC_CACHE
    if _NC_CACHE is not None:
        return _NC_CACHE
    nc = bacc.Bacc("TRN2", debug=False, target_bir_lowering=False,
                   num_devices=NCORES)
    ins = {
        "xT": nc.dram_tensor("xT", [C, T], BF, kind="ExternalInput").ap(),
        "wqT": nc.dram_tensor("wqT", [C, 512], BF, kind="ExternalInput").ap(),
        "wkT": nc.dram_tensor("wkT", [C, P], BF, kind="ExternalInput").ap(),
        "wvT": nc.dram_tensor("wvT", [C, P], BF, kind="ExternalInput").ap(),
        "woR": nc.dram_tensor("woR", [512, C], BF, kind="ExternalInput").ap(),
        "tri": nc.dram_tensor("tri", [P, P], BF, kind="ExternalInput").ap(),
        "iden": nc.dram_tensor("iden", [P, P], BF, kind="ExternalInput").ap(),
        "ones2": nc.dram_tensor("ones2", [2, P], BF, kind="ExternalInput").ap(),
        "vinit": nc.dram_tensor("vinit", [P, NTT, 130], BF,
                                kind="ExternalInput").ap(),
    }
    outs = {"y": nc.dram_tensor("y", [T, C], BF, kind="ExternalOutput").ap()}
    with tile.TileContext(nc) as tc:
        with tc.tile_pool(name="persist", bufs=1) as persist:
            tc._persist_pool = persist
            build_kernel(nc, tc, ins, outs)
    nc.compile()
    _NC_CACHE = nc
    return nc


def make_core_inputs(x, Wq, Wkv, Wo):
    """Host-side shard + pre-transpose + bf16 cast. Returns 8 in_maps."""
    x = np.asarray(x, np.float32)
    Wq = np.asarray(Wq, np.float32)
    Wkv = np.asarray(Wkv, np.float32)
    Wo = np.asarray(Wo, np.float32)
    tri = np.triu(np.ones((P, P), np.float32))   # keep t_local >= s_local
    iden = np.eye(P, dtype=np.float32)
    # ones2: partitions 0-63 of the broadcast get rowsum h0, 64-127 get h1
    ones2 = np.zeros((2, P), np.float32)
    ones2[0, 0:64] = 1.0
    ones2[1, 64:128] = 1.0
    in_maps = []
    for c in range(NCORES):
        b, gc = c // 4, c % 4
        xT = np.ascontiguousarray(x[b].T)                       # [C, T]
        Wq4 = Wq.reshape(HKV, G, D, C)[2 * gc:2 * gc + 2]       # [2, G, D, C]
        wqT = np.ascontiguousarray(
            np.transpose(Wq4, (1, 0, 2, 3)).reshape(512, C).T)  # [C, (g,kv,d)]
        wkT = np.ascontiguousarray(Wkv[2 * gc * 64:(2 * gc + 2) * 64].T)
        wvT = np.ascontiguousarray(
            Wkv[HKV * D + 2 * gc * 64:HKV * D + (2 * gc + 2) * 64].T)
        Wo4 = Wo.reshape(C, HKV, G, D)[:, 2 * gc:2 * gc + 2]    # [C, 2, G, D]
        woR = np.ascontiguousarray(
            np.transpose(Wo4, (2, 1, 3, 0)).reshape(512, C))    # [(g,kv,d), C]
        in_maps.append({
            "xT": xT.astype(BFNP), "wqT": wqT.astype(BFNP),
            "wkT": wkT.astype(BFNP), "wvT": wvT.astype(BFNP),
            "woR": woR.astype(BFNP), "tri": tri.astype(BFNP),
            "iden": iden.astype(BFNP), "ones2": ones2.astype(BFNP),
            "vinit": np.ones((P, NTT, 130), BFNP),
        })
    return in_maps


def kernel(x, Wq, Wkv, Wo, trace=False):
    nc = build_nc()
    in_maps = make_core_inputs(x, Wq, Wkv, Wo)
    res = run_bass_kernel_spmd(nc, in_maps, core_ids=list(range(NCORES)),
                               trace=trace)
    y = np.zeros((B, T, C), np.float32)
    for c in range(NCORES):
        y[c // 4] += np.asarray(res.results[c]["y"], np.float32)
    if trace:
        kernel.last_exec_time_ns = res.exec_time_ns
        kernel.last_results = res
    return y


# revision 33
# speedup vs baseline: 1.1716x; 1.1716x over previous
"""GroupedQueryAttention TRN2 kernel (bf16).

Sharding: 4-way tensor-parallel over heads x 2-way data-parallel over batch.
Core c handles batch b=c//4 and head-group gc=c%4 (kv heads {2gc, 2gc+1},
q heads (hkv, g) for g in 0..3 -> 8 q heads per core).

Per-core device program (T=2048, C=2048, D=64), all matmul operands bf16
(PSUM accumulation fp32):
  phase A (PE-bound ~82us): Q^T/K^T/V^T projections per 512-col block,
    V PE-transposed into V' [T,130] with ones cols (rowsum trick).
  phase B (Scalar exp-bound): S^T tiles = K_tile Q^T (causal widths),
    exp on ScalarE -> bf16, diag mask on GpSimd, O'^T accumulated on PE
    (row 64 = rowsum), normalize via ones2-matmul broadcast of rowsums +
    DVE reciprocal_approx_fast + DVE multiply.
  phase C (PE-bound): y_tile = O^T.T @ Wo_g, interleaved INTO phase B at
    (j,g) boundaries so PE fills exp-wait stalls with output-proj matmuls.
  Host sums the 4 TP partials per batch in fp32.

PSUM budget in B||C: ps 2x2 banks + po 2x1 + py/pru 2x1 = 8 banks.
"""

import numpy as np
import ml_dtypes

import concourse.bass as bass
import concourse.mybir as mybir
import concourse.tile as tile
from concourse import bacc
from concourse.bass_utils import run_bass_kernel_spmd

H, HKV, D, G = 32, 8, 64, 4
B, T, C = 2, 2048, 2048
P = 128
NCORES = 8
F32 = mybir.dt.float32
BF = mybir.dt.bfloat16
BFNP = ml_dtypes.bfloat16

NT = T // 512   # 4 q blocks of 512
NK = C // P     # 16 contraction tiles
NTT = T // P    # 16 key/row tiles

_NC_CACHE = None


def build_kernel(nc, tc, ins, outs):
    xTr = ins["xT"].rearrange("(ko p) t -> p ko t", p=P)
    y = outs["y"]
    EXP = mybir.ActivationFunctionType.Exp
    CPY = mybir.ActivationFunctionType.Copy

    # ---- persistent SBUF ----
    persist = tc._persist_pool
    qT_sb = persist.tile([P, G, T], BF, name="qT_sb", tag="qT_sb")
    kT_sb = persist.tile([P, T], BF, name="kT_sb", tag="kT_sb")
    v_sb = persist.tile([P, NTT, 130], BF, name="v_sb", tag="v_sb")
    oT_sb = persist.tile([P, G, T], BF, name="oT_sb", tag="oT_sb")
    wo_sb = persist.tile([P, G, C], BF, name="wo_sb", tag="wo_sb")
    tri_sb = persist.tile([P, P], BF, name="tri_sb", tag="tri_sb")
    iden_sb = persist.tile([P, P], BF, name="iden_sb", tag="iden_sb")
    ones2_sb = persist.tile([2, P], BF, name="ones2_sb", tag="ones2_sb")

    def emit_const_dmas():
        # deferred behind the first weight/x chunks: only needed ~20us in
        nc.sync.dma_start(tri_sb[:], ins["tri"][:])
        nc.sync.dma_start(iden_sb[:], ins["iden"][:])
        nc.sync.dma_start(ones2_sb[:], ins["ones2"][:])
        # ones columns of V' (64, 129 per key tile); data cols overwritten
        nc.sync.dma_start(v_sb[:], ins["vinit"][:])

    # ================= phase A: projections =================
    with (
        tc.tile_pool(name="wproj", bufs=1) as wpool,
        tc.tile_pool(name="xt", bufs=12) as xpool,
        tc.tile_pool(name="vtt", bufs=2) as vttpool,
        tc.tile_pool(name="pp", bufs=6, space="PSUM") as pp,
        tc.tile_pool(name="pvt", bufs=2, space="PSUM") as pvt,
    ):
        # weights and x in k-tile chunks as separate tiles: fine-grained
        # deps let the k=0 matmuls start after ~0.26 MB of DMA, not ~6 MB.
        # chunk c covers k tiles KCH[c]..KCH[c+1]
        KCH = [0, 1, 2, 4, 8, 12, 16]
        NCH = len(KCH) - 1
        kch = lambda k: next(c for c in range(NCH) if KCH[c + 1] > k)
        wq_t = [wpool.tile([P, KCH[c + 1] - KCH[c], 512], BF, name=f"wq_{c}")
                for c in range(NCH)]
        wk_sb = wpool.tile([P, NK, P], BF, name="wk_sb")
        wv_sb = wpool.tile([P, NK, P], BF, name="wv_sb")
        wqr = ins["wqT"].rearrange("(ko p) m -> p ko m", p=P)
        xt0 = [xpool.tile([P, KCH[c + 1] - KCH[c], 512], BF, tag="xt",
                          name=f"xt_0_{c}") for c in range(NCH)]
        for c in range(NCH):
            ks = slice(KCH[c], KCH[c + 1])
            nc.sync.dma_start(wq_t[c][:], wqr[:, ks, :])
            nc.sync.dma_start(xt0[c][:], xTr[:, ks, 0:512])
            if c == 0:
                nc.sync.dma_start(wk_sb[:],
                                  ins["wkT"].rearrange("(ko p) m -> p ko m", p=P))
                nc.sync.dma_start(wv_sb[:],
                                  ins["wvT"].rearrange("(ko p) m -> p ko m", p=P))
            if c == 1:
                emit_const_dmas()

        for tb in range(NT):
            ts = slice(tb * 512, (tb + 1) * 512)
            if tb == 0:
                xts = xt0
            else:
                xts = [xpool.tile([P, KCH[c + 1] - KCH[c], 512], BF, tag="xt",
                                  name=f"xt_{tb}_{c}") for c in range(NCH)]
                for c in range(NCH):
                    nc.sync.dma_start(xts[c][:], xTr[:, KCH[c]:KCH[c + 1], ts])
            psq = [pp.tile([P, 512], F32, tag="pp", name=f"psq_{tb}_{g}")
                   for g in range(G)]
            psk = pp.tile([P, 512], F32, tag="pp", name=f"psk_{tb}")
            psv = pp.tile([P, 512], F32, tag="pp", name=f"psv_{tb}")
            for k in range(NK):
                st, sp = (k == 0), (k == NK - 1)
                c = kch(k)
                xk = xts[c][:, k - KCH[c], :]
                for g in range(G):
                    nc.tensor.matmul(psq[g][:],
                                     wq_t[c][:, k - KCH[c], g * P:(g + 1) * P],
                                     xk, start=st, stop=sp)
                nc.tensor.matmul(psk[:], wk_sb[:, k, :], xk,
                                 start=st, stop=sp)
                nc.tensor.matmul(psv[:], wv_sb[:, k, :], xk,
                                 start=st, stop=sp)
            vtt = vttpool.tile([P, 512], BF, tag="vtt", name=f"vtt_{tb}")
            # copies split Scalar/DVE to shorten the per-tb tail
            nc.scalar.activation(qT_sb[:, 0, ts], psq[0][:], CPY)
            nc.scalar.activation(qT_sb[:, 1, ts], psq[1][:], CPY)
            nc.scalar.activation(kT_sb[:, ts], psk[:], CPY)
            nc.vector.tensor_copy(qT_sb[:, 2, ts], psq[2][:])
            nc.vector.tensor_copy(qT_sb[:, 3, ts], psq[3][:])
            nc.vector.tensor_copy(vtt[:], psv[:])
            # V^T -> V (PE transpose) into V' layout with ones cols
            for dd in range(4):
                tt = 4 * tb + dd
                pt_ = pvt.tile([P, P], BF, tag="pvt", name=f"pvt_{tt}")
                nc.tensor.transpose(pt_[:], vtt[:, dd * P:(dd + 1) * P],
                                    iden_sb[:])
                nc.vector.tensor_copy(v_sb[:, tt, 0:64], pt_[:, 0:64])
                nc.vector.tensor_copy(v_sb[:, tt, 65:129], pt_[:, 64:128])

    # ================= phase B || C =================
    for c4 in range(4):
        nc.sync.dma_start(
            wo_sb[:, c4, :],
            ins["woR"][128 * c4:128 * (c4 + 1), :].rearrange(
                "(m p) c -> p m c", p=P))

    with (
        tc.tile_pool(name="ps", bufs=2, space="PSUM") as pspool,
        tc.tile_pool(name="po", bufs=2, space="PSUM") as popool,
        tc.tile_pool(name="py", bufs=2, space="PSUM") as pypool,
        tc.tile_pool(name="ptp", bufs=6) as ptpool,
        tc.tile_pool(name="rs", bufs=8) as rspool,
        tc.tile_pool(name="rb", bufs=2) as rbpool,
        tc.tile_pool(name="ysb", bufs=2) as ypool,
    ):
        def emit_B_tiles(j, g, pool=None):
            """Scores + exp + mask + PV accumulation; returns po + rowsum
            copies (issued immediately so DVE overlaps the following C
            chunk)."""
            qs0 = j * 512
            pool, ptag = pool or (popool, "po")
            po = [pool.tile([P, 512], F32, tag=ptag, name=f"po_{j}_{g}_{h}")
                  for h in range(2)]
            ilast = 4 * j + 3
            for i in range(4 * j + 4):
                loc = max(0, P * i - qs0)
                ps = pspool.tile([P, 2, 512], F32, tag="ps",
                                 name=f"ps_{j}_{g}_{i}")
                pt_ = ptpool.tile([P, 2, 512], BF, tag="pt",
                                  name=f"pt_{j}_{g}_{i}")
                for h in range(2):
                    nc.tensor.matmul(
                        ps[:, h, loc:512],
                        kT_sb[h * 64:(h + 1) * 64, i * P:(i + 1) * P],
                        qT_sb[h * 64:(h + 1) * 64, g, qs0 + loc:qs0 + 512],
                        start=True, stop=True,
                    )
                nc.scalar.activation(pt_[:, :, loc:512], ps[:, :, loc:512],
                                     EXP, scale=0.125)
                if i >= 4 * j:  # diagonal tile: mask strict-lower triangle
                    nc.vector.tensor_mul(
                        pt_[:, :, loc:loc + P], pt_[:, :, loc:loc + P],
                        tri_sb[:, None, :].to_broadcast([P, 2, P]),
                    )
                for h in range(2):
                    nc.tensor.matmul(
                        po[h][0:65, loc:512],
                        v_sb[:, i, h * 65:h * 65 + 65],
                        pt_[:, h, loc:512],
                        start=(i == 0), stop=(i == ilast),
                    )
            rs = [rspool.tile([1, 512], BF, tag="rs", name=f"rs_{j}_{g}_{h}")
                  for h in range(2)]
            for h in range(2):
                nc.vector.tensor_copy(rs[h][:], po[h][64:65, :])
            return po, rs

        def emit_B_norm(j, g, po, rs, pool=None):
            """Broadcast rowsums (PE), reciprocal + normalize (DVE).
            Emitted after the interleaved C chunk so PE never waits on the
            DVE rowsum copies."""
            qs0 = j * 512
            pool, ptag = pool or (pypool, "py")
            pru = pool.tile([P, 512], F32, tag=ptag, name=f"pru_{j}_{g}")
            for h in range(2):
                nc.tensor.matmul(pru[h * 64:(h + 1) * 64, :], ones2_sb[0:1, 0:64],
                                 rs[h][:], start=True, stop=True)
            rb = rbpool.tile([P, 512], F32, tag="rb", name=f"rb_{j}_{g}")
            nc.vector.reciprocal_approx_fast(rb[:], pru[:])
            for h in range(2):
                nc.vector.tensor_mul(
                    oT_sb[h * 64:(h + 1) * 64, g, qs0:qs0 + 512],
                    po[h][0:64, :], rb[h * 64:(h + 1) * 64, :],
                )

        def emit_C_tt(tt, final=False):
            ysb = ypool.tile([P, T], BF, tag="ysb", name=f"y_{tt}")
            for cb in range(NT):
                py = pypool.tile([P, 512], F32, tag="py", name=f"py_{tt}_{cb}")
                for m in range(G):
                    nc.tensor.matmul(
                        py[:], oT_sb[:, m, tt * P:(tt + 1) * P],
                        wo_sb[:, m, cb * 512:(cb + 1) * 512],
                        start=(m == 0), stop=(m == G - 1),
                    )
                dst = ysb[:, cb * 512:(cb + 1) * 512]
                if final and cb % 2 == 0:
                    # drain phase: Scalar is idle, split copies + chunk DMA
                    nc.scalar.activation(dst, py[:], CPY)
                else:
                    nc.vector.tensor_copy(dst, py[:])
                if final and cb % 2 == 1:
                    nc.sync.dma_start(
                        y[tt * P:(tt + 1) * P, (cb - 1) * 512:(cb + 1) * 512],
                        ysb[:, (cb - 1) * 512:(cb + 1) * 512])
            if not final:
                nc.sync.dma_start(y[tt * P:(tt + 1) * P, :], ysb[:])

        # Normalize is delayed by one group: norm(prev) is emitted after
        # tiles(cur), so its rowsum copies (DVE) always finish while PE
        # runs the current group's score matmuls -- the broadcast matmul
        # never waits. C(j-1) chunks fill each (j,g) boundary.
        prev = None
        for j in range(NT):
            for g in range(G):
                cur = (j, g) + emit_B_tiles(j, g)
                if prev is not None:
                    emit_B_norm(*prev)
                if j >= 1:
                    emit_C_tt(4 * (j - 1) + g)
                prev = cur
        emit_B_norm(*prev)
        for dd in range(4):
            emit_C_tt(12 + dd, final=True)


def build_nc():
    global _NC_CACHE
    if _NC_CACHE is not None:
        return _NC_CACHE
    nc = bacc.Bacc("TRN2", debug=False, target_bir_lowering=False,
                   num_devices=NCORES)
    ins = {
        "xT": nc.dram_tensor("xT", [C, T], BF, kind="ExternalInput").ap(),
        "wqT": nc.dram_tensor("wqT", [C, 512], BF, kind="ExternalInput").ap(),
        "wkT": nc.dram_tensor("wkT", [C, P], BF, kind="ExternalInput").ap(),
        "wvT": nc.dram_tensor("wvT", [C, P], BF, kind="ExternalInput").ap(),
        "woR": nc.dram_tensor("woR", [512, C], BF, kind="ExternalInput").ap(),
        "tri": nc.dram_tensor("tri", [P, P], BF, kind="ExternalInput").ap(),
        "iden": nc.dram_tensor("iden", [P, P], BF, kind="ExternalInput").ap(),
        "ones2": nc.dram_tensor("ones2", [2, P], BF, kind="ExternalInput").ap(),
        "vinit": nc.dram_tensor("vinit", [P, NTT, 130], BF,
                                kind="ExternalInput").ap(),
    }
    outs = {"y": nc.dram_tensor("y", [T, C], BF, kind="ExternalOutput").ap()}
    with tile.TileContext(nc) as tc:
        with tc.tile_pool(name="persist", bufs=1) as persist:
            tc._persist_pool = persist
            build_kernel(nc, tc, ins, outs)
    nc.compile()
    _NC_CACHE = nc
    return nc


def make_core_inputs(x, Wq, Wkv, Wo):
    """Host-side shard + pre-transpose + bf16 cast. Returns 8 in_maps."""
    x = np.asarray(x, np.float32)
    Wq = np.asarray(Wq, np.float32)
    Wkv = np.asarray(Wkv, np.float32)
    Wo = np.asarray(Wo, np.float32)
    tri = np.triu(np.ones((P, P), np.float32))   # keep t_local >= s_local
    iden = np.eye(P, dtype=np.float32)
    # ones2: partitions 0-63 of the broadcast get rowsum h0, 64-127 get h1
    ones2 = np.zeros((2, P), np.float32)
    ones2[0, 0:64] = 1.0
    ones2[1, 64:128] = 1.0
    in_maps = []
    for c in range(NCORES):
        b, gc = c // 4, c % 4
        xT = np.ascontiguousarray(x[b].T)                       # [C, T]
        Wq4 = Wq.reshape(HKV, G, D, C)[2 * gc:2 * gc + 2]       # [2, G, D, C]
        wqT = np.ascontiguousarray(
            np.transpose(Wq4, (1, 0, 2, 3)).reshape(512, C).T)  # [C, (g,kv,d)]
        wkT = np.ascontiguousarray(Wkv[2 * gc * 64:(2 * gc + 2) * 64].T)
        wvT = np.ascontiguousarray(
            Wkv[HKV * D + 2 * gc * 64:HKV * D + (2 * gc + 2) * 64].T)
        Wo4 = Wo.reshape(C, HKV, G, D)[:, 2 * gc:2 * gc + 2]    # [C, 2, G, D]
        woR = np.ascontiguousarray(
            np.transpose(Wo4, (2, 1, 3, 0)).reshape(512, C))    # [(g,kv,d), C]
        in_maps.append({
            "xT": xT.astype(BFNP), "wqT": wqT.astype(BFNP),
            "wkT": wkT.astype(BFNP), "wvT": wvT.astype(BFNP),
            "woR": woR.astype(BFNP), "tri": tri.astype(BFNP),
            "iden": iden.astype(BFNP), "ones2": ones2.astype(BFNP),
            "vinit": np.ones((P, NTT, 130), BFNP),
        })
    return in_maps


def kernel(x, Wq, Wkv, Wo, trace=False):
    nc = build_nc()
    in_maps = make_core_inputs(x, Wq, Wkv, Wo)
    res = run_bass_kernel_spmd(nc, in_maps, core_ids=list(range(NCORES)),
                               trace=trace)
    y = np.zeros((B, T, C), np.float32)
    for c in range(NCORES):
        y[c // 4] += np.asarray(res.results[c]["y"], np.float32)
    if trace:
        kernel.last_exec_time_ns = res.exec_time_ns
        kernel.last_results = res
    return y
